# revision 2
# baseline (speedup 1.0000x reference)
"""Trainium2 Bass kernel for nn_ComplexAttention (B=8, C=512, H=W=32, HEADS=8).

Strategy
--------
Data-parallel over batch: one batch element per NeuronCore (8 cores), no
collectives.  Host-side algebraic fusion shrinks the per-core work:

  reference:  Q = R_q Wq Z,  K = R_k Wk Z,  V = R_v Wv Z   (complex, [C,T])
              S = Re(Q^H K)/sqrt(dh),  causal softmax -> A
              out = R_o Wo (V A^T)
  fused:      M = Wq^T diag(e^{i(phi_k-phi_q)}) Wk / sqrt(dh)   (host, f64)
              N = diag(e^{i phi_o}) Wo diag(e^{i phi_v}) Wv     (host, f64)
              Y = M Z            (channel-major [C,T])
              E = exp(mask(Y^H Z))  computed TRANSPOSED: E[u,t], u=key, t=query
              l = ones^T E       (column sums via M=1 matmuls on the PE)
              U = N Z            (token-major [T,C])
              out[:, t] = (U^T E)[:, t] / l[t]   (1/l folded into the
                          PSUM->SBUF drain as a tensor_tensor multiply with a
                          gpsimd-partition-broadcast row of reciprocals)

Computing the scores transposed (lhsT=Y tiles instead of lhsT=Z tiles) means
the attention matrix comes out of the PE already in the layout the output
matmul wants: the 36 PE transposes + 36 DVE copies + 8 row-normalization
passes of the non-transposed scheme all disappear.

Everything runs in bf16 (PE streams 1 col/cycle for both f32r and bf16, but
bf16 halves HBM+SBUF traffic and enables fast weight load).  Measured rel
err ~8e-3 vs the f64 oracle (tolerance 2e-2).

Schedule notes:
 - NWARM dummy matmuls on a memset tile run while the first input DMAs are
   in flight, so the PE's HAM clock gate reaches 8/8 (2.4 GHz) before real
   work starts instead of ~9us into it.
 - phases: Y half0 -> scores(q=0) x u(0..3) -> out chunk 0 -> Y half1 ->
   scores(q=1) x u(4..7) -> out chunk 1.  The big out chunk is last but its
   stores stream per m-pair; the final DMA is a single 256KB transfer.
 - input DMAs are emitted just-in-time (per c-tile hooks) so the tile
   framework's watermark-style sem waits don't over-cover; stores go on the
   gpsimd queue so they never delay a load descriptor.
"""

import math

import numpy as np

import concourse.mybir as mybir
import concourse.tile as tile
from concourse import bacc
from concourse.bass_utils import run_bass_kernel_spmd

B, C, HH, WW = 8, 512, 32, 32
T = HH * WW          # 1024 tokens
DH = C // 8          # head dim (scale only)
P = 128
CT = C // P          # 4 channel tiles
TT = T // P          # 8 token tiles
NEG = -1.0e30
NWARM = 20           # PE warm-up matmuls (HAM un-throttle before real work)

f32 = mybir.dt.float32
bf16 = mybir.dt.bfloat16


_CACHE: dict = {}


def _get_program(has_imag: bool):
    key = has_imag
    if key not in _CACHE:
        _CACHE[key] = _build_program(has_imag)
    return _CACHE[key]


def _build_program(has_imag: bool):
    nc = bacc.Bacc("TRN2", target_bir_lowering=False, debug=False)

    sdt = bf16
    zre_d = nc.dram_tensor("zre", [C, T], sdt, kind="ExternalInput").ap()
    zim_d = nc.dram_tensor("zim", [C, T], sdt, kind="ExternalInput").ap()
    mtre_d = nc.dram_tensor("mtre", [C, C], sdt, kind="ExternalInput").ap()
    ntre_d = nc.dram_tensor("ntre", [C, C], sdt, kind="ExternalInput").ap()
    if has_imag:
        mtim_d = nc.dram_tensor("mtim", [C, C], sdt, kind="ExternalInput").ap()
        mtimn_d = nc.dram_tensor("mtimn", [C, C], sdt,
                                 kind="ExternalInput").ap()
        ntim_d = nc.dram_tensor("ntim", [C, C], sdt, kind="ExternalInput").ap()
        ntimn_d = nc.dram_tensor("ntimn", [C, C], sdt,
                                 kind="ExternalInput").ap()
    ones_d = nc.dram_tensor("ones", [P, 1], sdt, kind="ExternalInput").ap()
    m128_d = nc.dram_tensor("m128", [P, P], f32, kind="ExternalInput").ap()
    m256_d = nc.dram_tensor("m256", [P, 256], f32, kind="ExternalInput").ap()
    outre_d = nc.dram_tensor("outre", [C, T], f32, kind="ExternalOutput").ap()
    outim_d = nc.dram_tensor("outim", [C, T], f32, kind="ExternalOutput").ap()

    with tile.TileContext(nc) as tc:
        with (
            tc.tile_pool(name="const", bufs=1) as cp,
            tc.tile_pool(name="work", bufs=4) as wp,
            tc.tile_pool(name="psmm", bufs=6, space="PSUM") as pmm,
            tc.tile_pool(name="psl", bufs=1, space="PSUM") as psl,
        ):
            # -- constants + warm-up ---------------------------------------
            ones = cp.tile([P, 1], sdt, tag="ones", name="ones")
            nc.gpsimd.dma_start(out=ones, in_=ones_d)
            m128 = cp.tile([P, P], f32, tag="m128", name="m128")
            nc.gpsimd.dma_start(out=m128, in_=m128_d)
            m256 = cp.tile([P, 256], f32, tag="m256", name="m256")
            nc.gpsimd.dma_start(out=m256, in_=m256_d)

            warm = cp.tile([P, 512], sdt, tag="warm", name="warm")
            nc.vector.memset(warm, 0.0)
            for _ in range(NWARM):
                wps = pmm.tile([P, 512], f32, tag="mm", name="psmm")
                nc.tensor.matmul(wps, warm[:, :P], warm, start=True,
                                 stop=True)

            # -- persistent tiles ------------------------------------------
            mtre = [cp.tile([P, C], sdt, tag=f"mtre{c}", name=f"mtre{c}")
                    for c in range(CT)]
            ntre = [cp.tile([P, C], sdt, tag=f"ntre{c}", name=f"ntre{c}")
                    for c in range(CT)]
            zre_h = [[cp.tile([P, 512], sdt, tag=f"zre{c}_{h}",
                              name=f"zre{c}_{h}") for c in range(CT)]
                     for h in range(2)]
            zim_h = [[cp.tile([P, 512], sdt, tag=f"zim{c}_{h}",
                              name=f"zim{c}_{h}") for c in range(CT)]
                     for h in range(2)]
            yre = [[cp.tile([P, 512], sdt, tag=f"yre{c}_{h}",
                            name=f"yre{c}_{h}") for h in range(2)]
                   for c in range(CT)]
            yim = [[cp.tile([P, 512], sdt, tag=f"yim{c}_{h}",
                            name=f"yim{c}_{h}") for h in range(2)]
                   for c in range(CT)]
            ure = [cp.tile([P, C], sdt, tag=f"ure{j}", name=f"ure{j}")
                   for j in range(TT)]
            uim = [cp.tile([P, C], sdt, tag=f"uim{j}", name=f"uim{j}")
                   for j in range(TT)]
            if has_imag:
                mtim = [cp.tile([P, C], sdt, tag=f"mtim{c}") for c in range(CT)]
                mtimn = [cp.tile([P, C], sdt, tag=f"mtimn{c}")
                         for c in range(CT)]
                ntim = [cp.tile([P, C], sdt, tag=f"ntim{c}") for c in range(CT)]
                ntimn = [cp.tile([P, C], sdt, tag=f"ntimn{c}")
                         for c in range(CT)]

            lps = [psl.tile([1, 512], f32, tag=f"l{q}", name=f"l{q}")
                   for q in range(2)]
            sxs: dict = {}
            rlb: dict = {}

            # -- emit helpers ----------------------------------------------
            def emit_y_half(dst, terms, h, load_hook=None):
                pss = [pmm.tile([P, 512], f32, tag="mm", name="psmm")
                       for _ in range(CT)]
                nterm = len(terms)
                for t_i, (w, zh) in enumerate(terms):
                    for c in range(CT):
                        if load_hook is not None:
                            load_hook(c, t_i)
                        for m in range(CT):
                            nc.tensor.matmul(
                                pss[m], w[c][:, m * P:(m + 1) * P], zh[h][c],
                                start=(t_i == 0 and c == 0),
                                stop=(t_i == nterm - 1 and c == CT - 1))
                for m in range(CT):
                    nc.vector.tensor_copy(out=dst[m][h], in_=pss[m])

            def emit_u(j, dst, terms):
                usl = slice((j % 4) * P, (j % 4 + 1) * P)
                ps = pmm.tile([P, 512], f32, tag="mm", name="psmm")
                nacc = len(terms) * CT
                k = 0
                for zh, w in terms:
                    for c in range(CT):
                        nc.tensor.matmul(ps, zh[j // 4][c][:, usl], w[c],
                                         start=(k == 0), stop=(k == nacc - 1))
                        k += 1
                nc.vector.tensor_copy(out=dst[j], in_=ps)

            def scores_block(j, q, score_terms):
                """E[u-tile j, t-chunk q] = exp(mask(S^T block)) in SBUF."""
                d = j * P - q * 512
                lo = 0 if d <= P else d - P
                usl = slice((j % 4) * P, (j % 4 + 1) * P)
                ps = pmm.tile([P, 512], f32, tag="mm", name="psmm")
                k = 0
                nacc = len(score_terms) * CT
                for zh, y in score_terms:
                    for c in range(CT):
                        nc.tensor.matmul(
                            ps[:, lo:512], y[c][j // 4][:, usl],
                            zh[q][c][:, lo:512],
                            start=(k == 0), stop=(k == nacc - 1))
                        k += 1
                if d == 0:
                    nc.vector.tensor_add(out=ps[:, 0:P], in0=ps[:, 0:P],
                                         in1=m128)
                elif d > 0:
                    nc.vector.tensor_add(out=ps[:, d - P:d + P],
                                         in0=ps[:, d - P:d + P], in1=m256)
                sx = cp.tile([P, 512], sdt, tag=f"sx{j}_{q}",
                             name=f"sx{j}_{q}")
                nc.scalar.activation(
                    out=sx[:, lo:512], in_=ps[:, lo:512],
                    func=mybir.ActivationFunctionType.Exp)
                sxs[(j, q)] = (sx, lo)

            def emit_ones(j, q, start=False, stop=False):
                sx, lo = sxs[(j, q)]
                nc.tensor.matmul(lps[q][:, lo:512], ones, sx[:, lo:512],
                                 start=start, stop=stop,
                                 skip_group_check=True)

            def emit_recip(q):
                rl = cp.tile([1, 512], f32, tag=f"rl{q}", name=f"rl{q}")
                nc.vector.reciprocal(out=rl, in_=lps[q])
                rb = cp.tile([P, 512], f32, tag=f"rlb{q}", name=f"rlb{q}")
                nc.gpsimd.partition_broadcast(rb, rl)
                rlb[q] = rb

            outre_v = outre_d.rearrange("(m p) t -> p m t", p=P)
            outim_v = outim_d.rearrange("(m p) t -> p m t", p=P)

            def out_group(n, half, mh, split_dma=False, mid_hook=None):
                u = ure if half == 0 else uim
                dview = outre_v if half == 0 else outim_v
                tsl = slice(n * 512, (n + 1) * 512)
                js = list(range(4 * (n + 1)))
                osb = wp.tile([P, 2, 512], f32, tag="osb", name="osb")
                for mi in range(2):
                    m = 2 * mh + mi
                    msl = slice(m * P, (m + 1) * P)
                    ps = pmm.tile([P, 512], f32, tag="mm", name="psmm")
                    for ji, j in enumerate(js):
                        lo = max(0, j * P - n * 512)
                        nc.tensor.matmul(
                            ps[:, lo:512], u[j][:, msl],
                            sxs[(j, n)][0][:, lo:512],
                            start=(ji == 0), stop=(ji == len(js) - 1))
                    if mi == 0 and mid_hook is not None:
                        mid_hook()
                    nc.vector.tensor_mul(out=osb[:, mi, :], in0=ps,
                                         in1=rlb[n])
                    if split_dma:
                        nc.gpsimd.dma_start(out=dview[:, m:m + 1, tsl],
                                            in_=osb[:, mi:mi + 1, :])
                if not split_dma:
                    nc.gpsimd.dma_start(out=dview[:, 2 * mh:2 * mh + 2, tsl],
                                        in_=osb)

            # -- load hooks (JIT DMA emission on the sync queue) -----------
            def hook_mtre_zre(c, t_i):
                if t_i == 0:
                    nc.sync.dma_start(out=mtre[c],
                                      in_=mtre_d[c * P:(c + 1) * P, :])
                    nc.sync.dma_start(out=zre_h[0][c],
                                      in_=zre_d[c * P:(c + 1) * P, 0:512])

            def hook_zim0(c, t_i):
                if t_i == 0:
                    nc.sync.dma_start(out=zim_h[0][c],
                                      in_=zim_d[c * P:(c + 1) * P, 0:512])

            def mk_half_hook(tiles_h, dram, h):
                def hook(c, t_i):
                    if t_i == 0:
                        nc.sync.dma_start(
                            out=tiles_h[h][c],
                            in_=dram[c * P:(c + 1) * P, h * 512:(h + 1) * 512])
                return hook

            # -- phase plan ------------------------------------------------
            if not has_imag:
                yterms_re = [(mtre, zre_h)]
                yterms_im = [(mtre, zim_h)]
                uterms_re = [(zre_h, ntre)]
                uterms_im = [(zim_h, ntre)]
            else:
                yterms_re = [(mtre, zre_h), (mtimn, zim_h)]
                yterms_im = [(mtre, zim_h), (mtim, zre_h)]
                uterms_re = [(zre_h, ntre), (zim_h, ntimn)]
                uterms_im = [(zim_h, ntre), (zre_h, ntim)]
            score_terms = ((zre_h, yre), (zim_h, yim))

            if has_imag:
                for c in range(CT):
                    hook_mtre_zre(c, 0)
                for c in range(CT):
                    hook_zim0(c, 0)
                for tiles, dram in ((mtim, mtim_d), (mtimn, mtimn_d),
                                    (ntim, ntim_d), (ntimn, ntimn_d)):
                    for c in range(CT):
                        nc.sync.dma_start(out=tiles[c],
                                          in_=dram[c * P:(c + 1) * P, :])
                hooks = [None, None, None, None]
            else:
                hooks = [hook_mtre_zre, hook_zim0,
                         mk_half_hook(zre_h, zre_d, 1),
                         mk_half_hook(zim_h, zim_d, 1)]

            # Y half 0
            emit_y_half(yre, yterms_re, 0, hooks[0])
            emit_y_half(yim, yterms_im, 0, hooks[1])
            for c in range(CT):
                nc.sync.dma_start(out=ntre[c],
                                  in_=ntre_d[c * P:(c + 1) * P, :])

            # scores chunk 0 (j=0..3) interleaved with U j=0..3
            for j in range(4):
                scores_block(j, 0, score_terms)
                emit_u(j, ure, uterms_re)
                emit_u(j, uim, uterms_im)
                emit_ones(j, 0, start=(j == 0), stop=(j == 3))
            emit_recip(0)

            # out chunk 0 (t 0..511)
            out_group(0, 0, 0)
            out_group(0, 0, 1)
            out_group(0, 1, 0)
            out_group(0, 1, 1)

            # Y half 1
            emit_y_half(yre, yterms_re, 1, hooks[2])
            emit_y_half(yim, yterms_im, 1, hooks[3])

            # scores chunk 1: j=0..3 (full) with U j=4..7, then j=4..7
            for j in range(4):
                scores_block(j, 1, score_terms)
                emit_u(j + 4, ure, uterms_re)
                emit_u(j + 4, uim, uterms_im)
                emit_ones(j, 1, start=(j == 0))
            scores_block(4, 1, score_terms)
            scores_block(5, 1, score_terms)
            emit_ones(4, 1)
            scores_block(6, 1, score_terms)
            emit_ones(5, 1)
            scores_block(7, 1, score_terms)
            emit_ones(6, 1)

            # out chunk 1 (t 512..1023); flush last ones + recip after the
            # first psum group's matmuls (exp(7,1) is done by then)
            def late_l1():
                emit_ones(7, 1, stop=True)
                emit_recip(1)

            out_group(1, 0, 0, mid_hook=late_l1)
            out_group(1, 0, 1)
            out_group(1, 1, 0)
            out_group(1, 1, 1, split_dma=True)

    nc.compile()
    return nc


def _prep_weights(Wq, phi_q, Wk, phi_k, Wv, phi_v, Wo, phi_o):
    Wq, Wk, Wv, Wo = (np.asarray(w, np.float64) for w in (Wq, Wk, Wv, Wo))
    pq, pk, pv, po = (np.asarray(p, np.float64)
                      for p in (phi_q, phi_k, phi_v, phi_o))
    M = (Wq.T @ (np.exp(1j * (pk - pq))[:, None] * Wk)) / math.sqrt(DH)
    N = (np.exp(1j * po)[:, None] * Wo) @ (np.exp(1j * pv)[:, None] * Wv)
    has_imag = not (np.allclose(M.imag, 0.0) and np.allclose(N.imag, 0.0))
    return M, N, has_imag


def _consts(has_imag, M, N):
    import ml_dtypes
    snp = ml_dtypes.bfloat16
    consts = {
        "mtre": np.ascontiguousarray(M.real.T.astype(snp)),
        "ntre": np.ascontiguousarray(N.real.T.astype(snp)),
        "ones": np.ones((P, 1), snp),
        "m128": np.tril(np.full((P, P), NEG, np.float32), -1),
        "m256": np.concatenate(
            [np.full((P, P), NEG, np.float32),
             np.tril(np.full((P, P), NEG, np.float32), -1)], axis=1),
    }
    if has_imag:
        mtim = np.ascontiguousarray(M.imag.T.astype(snp))
        ntim = np.ascontiguousarray(N.imag.T.astype(snp))
        consts.update(mtim=mtim, mtimn=-mtim, ntim=ntim, ntimn=-ntim)
    return consts


def kernel(z_re, z_im, Wq, phi_q, Wk, phi_k, Wv, phi_v, Wo, phi_o):
    import ml_dtypes
    snp = ml_dtypes.bfloat16
    z_re = np.ascontiguousarray(np.asarray(z_re, np.float32).astype(snp))
    z_im = np.ascontiguousarray(np.asarray(z_im, np.float32).astype(snp))
    M, N, has_imag = _prep_weights(Wq, phi_q, Wk, phi_k, Wv, phi_v, Wo, phi_o)
    consts = _consts(has_imag, M, N)
    nc = _get_program(has_imag)
    in_maps = [
        dict(consts, zre=z_re[b].reshape(C, T), zim=z_im[b].reshape(C, T))
        for b in range(B)
    ]
    res = run_bass_kernel_spmd(nc, in_maps, list(range(B)))
    out_re = np.stack([res.results[b]["outre"].reshape(C, HH, WW)
                       for b in range(B)])
    out_im = np.stack([res.results[b]["outim"].reshape(C, HH, WW)
                       for b in range(B)])
    return out_re, out_im
